# revision 52
# baseline (speedup 1.0000x reference)
"""Trainium2 Bass kernel for multi-head causal attention with RoPE.

Model (per reference):
  B=2, S=2048, D=4096, H=32 heads, HD=128.
  out = softmax(rope(x@wq) @ rope(x@wk)^T / sqrt(HD) + mask) @ (x@wv) @ wo

Sharding: tensor-parallel over heads. Core c in 0..7 owns heads 4c..4c+3:
wq/wk/wv column-sharded, wo row-sharded; each core produces a full-shape
partial output and the host sums the 8 partials (the all-reduce).

Per-core pipeline:
  Phase B:  V projection from an fp16 copy of x with fp16 wv (values
            only touch the PV average, so fp16 error stays tiny);
            spilled to DRAM in [128, kt, 512] layout; V-phase DMA slack
            prefetches the full fp32r wq/wk into SBUF so the Q/K phase
            starts hot.
  Phase A:  Q,K projections in transposed [HD, tok] layout (lhsT =
            weight tile, rhs = xT tile), fp32r. RoPE in rotate-half
            form: PSUM drains split ACT (q banks) / DVE (k banks), sin
            products swapped in place by three block SBUF DMAs on the
            idle SWDGE queue, cos products multiplied into the PSUM
            copies in place, one fp16 add per head-tile rounds the rope
            result once; two block spills per chunk (SWDGE). All
            loose-deadline DMAs ride the gpsimd queue so the sync
            queue streams x tiles and the scalar queue only drains
            PSUM - neither ever blocks on a semaphore.
  Phase 2:  per (batch,head): S^T[k,q] = K-tile' @ Q chunk (fp16),
            512 wide; causal mask added only on diagonal tiles; exp on
            ScalarE with fused 1/sqrt(HD) scale (scores are bounded so
            no max subtraction); column sums via ones-vector matmul and
            fp32r PV in PSUM, software-pipelined one k-tile behind the
            score matmuls; sums+PV PSUM pair double-buffered across
            q-chunks; normalization after PV via gpsimd partition-
            broadcast reciprocal, output stored fp16. wo (fp16)
            streams in under phase-2 DMA slack.
  Phase 3:  out_partial[tok,:] += hoT_h' @ wo_rows_h (fp16 inputs)
            accumulated over the 4 local heads in PSUM, two 512-col
            groups per 1024-wide output store.
"""

import sys

if "/opt/trn_rl_repo" not in sys.path:
    sys.path.insert(0, "/opt/trn_rl_repo")

import math

import numpy as np

B, S, D, H = 2, 2048, 4096, 32
HD = D // H          # 128
HLOC = 4             # heads per core
NC = 8               # cores
TOK = B * S          # 4096
CH = TOK // 512      # 8 token chunks of 512
DKT = D // 128       # 32 contraction tiles
QC = S // 512        # 4 q-chunks per sequence
KT = S // 128        # 16 k-tiles per sequence
ISQRT = 1.0 / math.sqrt(HD)

_CACHE = {}


def _build(causal: bool, nrep: int = 1):
    import concourse.bacc as bacc
    import concourse.tile as tile
    from concourse import mybir

    F32 = mybir.dt.float32
    F32R = mybir.dt.float32r
    F16 = mybir.dt.float16
    EXP = mybir.ActivationFunctionType.Exp

    nc = bacc.Bacc("TRN2", target_bir_lowering=False, debug=False, num_devices=NC)

    xt_d = nc.dram_tensor("xt", [DKT, 128, TOK], F32R, kind="ExternalInput")
    xt16_d = nc.dram_tensor("xt16", [DKT, 128, TOK], F16, kind="ExternalInput")
    wq_d = nc.dram_tensor("wq", [128, DKT, 512], F32R, kind="ExternalInput")
    wk_d = nc.dram_tensor("wk", [128, DKT, 512], F32R, kind="ExternalInput")
    wv_d = nc.dram_tensor("wv16", [128, DKT, 512], F16, kind="ExternalInput")
    wo_d = nc.dram_tensor("wo16", [128, HLOC, D], F16, kind="ExternalInput")
    css_d = nc.dram_tensor("css", [128, 2, S], F16, kind="ExternalInput")
    if causal:
        mk_d = nc.dram_tensor("maskd", [128, 4, 512], F16, kind="ExternalInput")
    else:
        mk_d = nc.dram_tensor("maskf", [KT, 128, S], F32, kind="ExternalInput")
    out_d = nc.dram_tensor("out", [TOK, D], F32, kind="ExternalOutput")

    # DRAM scratch for projected Q/K/V
    qdr = nc.dram_tensor("qdr", [128, HLOC, B, S], F16)
    kdr = nc.dram_tensor("kdr", [128, HLOC, B, S], F16)
    vdr = {b: nc.dram_tensor(f"vdr{b}", [128, KT, 512], F32R) for b in range(B)}

    with tile.TileContext(nc) as tc:
        for _it in range(nrep):
            _emit_iter(nc, tc, _it, causal, xt_d, xt16_d, wq_d, wk_d, wv_d,
                       wo_d, css_d, mk_d, out_d, qdr, kdr, vdr,
                       F32, F32R, F16, EXP)

    nc.compile()
    return nc


def _emit_iter(nc, tc, it, causal, xt_d, xt16_d, wq_d, wk_d, wv_d, wo_d,
               css_d, mk_d, out_d, qdr, kdr, vdr, F32, F32R, F16, EXP):

    with tc.tile_pool(name=f"hb0{it}", bufs=1) as hb0p:
      with tc.tile_pool(name=f"w1{it}", bufs=1) as w1:
        wq_sb = w1.tile([128, DKT, 512], F32R, tag="wq")
        wk_sb = w1.tile([128, DKT, 512], F32R, tag="wk")
        if causal:
            mk_sb = hb0p.tile([128, 4, 512], F16, tag="mkd")
            nc.scalar.dma_start(out=mk_sb, in_=mk_d.ap())
        else:
            mk_sb = None

        # ------------- Phase B: V projection (fp16) + wq/wk prefetch ----
        with (
            tc.tile_pool(name=f"w2{it}", bufs=1) as w2,
            tc.tile_pool(name=f"xt2{it}", bufs=3) as xt2,
            tc.tile_pool(name=f"vcp{it}", bufs=3) as vcp,
            tc.tile_pool(name=f"ps2{it}", bufs=2, space="PSUM") as ps2,
        ):
            wv_sb = w2.tile([128, DKT, 512], F16, tag="wv")
            for ch in range(CH):
                b = ch // QC
                if ch >= 2:
                    # spread the 16MB wq/wk prefetch over chunks 2..7
                    # (chunk 0-1 DMA is busy with wv + x16)
                    lo = (ch - 2) * 6
                    hi = min(DKT, lo + 6)
                    nc.scalar.dma_start(out=wq_sb[:, lo:hi, :],
                                        in_=wq_d.ap()[:, lo:hi, :])
                    nc.scalar.dma_start(out=wk_sb[:, lo:hi, :],
                                        in_=wk_d.ap()[:, lo:hi, :])
                vps = [ps2.tile([128, 512], F32, name=f"vps{t}", tag=f"v{t}")
                       for t in range(4)]
                for dk4 in range(DKT // 4):
                    if ch == 0:
                        nc.scalar.dma_start(
                            out=wv_sb[:, 4 * dk4:4 * dk4 + 4, :],
                            in_=wv_d.ap()[:, 4 * dk4:4 * dk4 + 4, :],
                        )
                    xt = xt2.tile([128, 4, 512], F16, name="xt", tag="xt")
                    nc.sync.dma_start(
                        out=xt,
                        in_=xt16_d.ap()[4 * dk4:4 * dk4 + 4, :,
                                        ch * 512:(ch + 1) * 512]
                        .rearrange("k p t -> p k t"),
                    )
                    for i in range(4):
                        dk = 4 * dk4 + i
                        for t in range(4):
                            nc.tensor.matmul(
                                vps[t], xt[:, i, t * 128:(t + 1) * 128],
                                wv_sb[:, dk, :],
                                start=(dk == 0), stop=(dk == DKT - 1),
                            )
                for t in range(4):
                    vc = vcp.tile([128, 512], F32R, tag="vc")
                    if t % 2 == 0:
                        nc.scalar.copy(vc, vps[t])
                    else:
                        nc.vector.tensor_copy(vc, vps[t])
                    nc.gpsimd.dma_start(
                        out=vdr[b].ap()[:, (ch % QC) * 4 + t, :], in_=vc
                    )

        # ------------- Phase A: Q,K projections + RoPE ------------------
        if True:
            def load_hb(i, tiles, pool, skip_v=False):
                b, h = divmod(i, HLOC)
                qT = pool.tile([128, S], F16, name=f"qT{it}_{i}", tag="qT")
                kT = pool.tile([128, S], F16, name=f"kT{it}_{i}", tag="kT")
                eng = nc.gpsimd if pool is hb0p else nc.sync
                eng.dma_start(out=qT, in_=qdr.ap()[:, h, b, :])
                eng.dma_start(out=kT, in_=kdr.ap()[:, h, b, :])
                vT = None
                if not skip_v:
                    vT = pool.tile([128, KT, 128], F32R, name=f"vT{it}_{i}",
                                   tag="vT")
                    eng.dma_start(out=vT,
                                  in_=vdr[b].ap()[:, :, h * 128:(h + 1) * 128])
                tiles[i] = (qT, kT, vT)

            tiles = {}
            with (
                tc.tile_pool(name=f"xt1{it}", bufs=3) as xt1,
                tc.tile_pool(name=f"css{it}", bufs=1) as cssp,
                tc.tile_pool(name=f"rope{it}", bufs=1) as rope,
                tc.tile_pool(name=f"o16p{it}", bufs=1) as o16p,
                tc.tile_pool(name=f"ps1{it}", bufs=1, space="PSUM") as ps1,
            ):
                for ch in range(CH):
                    b, s0 = ch // QC, (ch % QC) * 512
                    css_sb = cssp.tile([128, 2, 512], F16, name="css_c",
                                       tag="css_c")
                    nc.gpsimd.dma_start(out=css_sb,
                                        in_=css_d.ap()[:, :, s0:s0 + 512])
                    cs_sb = css_sb[:, 0, :]
                    ss_sb = css_sb[:, 1, :]
                    qps = [ps1.tile([128, 512], F32, name=f"qps{h}", tag=f"q{h}")
                           for h in range(HLOC)]
                    kps = [ps1.tile([128, 512], F32, name=f"kps{h}", tag=f"k{h}")
                           for h in range(HLOC)]
                    for dk2 in range(DKT // 2):
                        xt = xt1.tile([128, 2, 512], F32R, name="xt", tag="xt")
                        nc.sync.dma_start(
                            out=xt,
                            in_=xt_d.ap()[2 * dk2:2 * dk2 + 2, :,
                                          ch * 512:(ch + 1) * 512]
                            .rearrange("k p t -> p k t"),
                        )
                        for i in range(2):
                            dk = 2 * dk2 + i
                            for h in range(HLOC):
                                nc.tensor.matmul(
                                    qps[h], wq_sb[:, dk, h * 128:(h + 1) * 128],
                                    xt[:, i, :],
                                    start=(dk == 0), stop=(dk == DKT - 1),
                                )
                            for h in range(HLOC):
                                nc.tensor.matmul(
                                    kps[h], wk_sb[:, dk, h * 128:(h + 1) * 128],
                                    xt[:, i, :],
                                    start=(dk == 0), stop=(dk == DKT - 1),
                                )
                    # Epilogue: drain q banks on ACT, k banks on DVE.
                    pcs = []
                    for h in range(HLOC):
                        pq = rope.tile([128, 512], F32, name="pcA", tag="pcA",
                                       bufs=4)
                        nc.scalar.copy(pq, qps[h])
                        pcs.append(pq)
                    for h in range(HLOC):
                        pk = rope.tile([128, 512], F32, name="pcB", tag="pcB",
                                       bufs=4)
                        nc.vector.tensor_copy(pk, kps[h])
                        pcs.append(pk)
                    s1b = rope.tile([128, 8, 512], F16, name="s1b", tag="s1b",
                                    bufs=1)
                    tmp = rope.tile([64, 4, 512], F16, name="swt", tag="swt",
                                    bufs=1)
                    o16 = o16p.tile([128, 8, 512], F16, name="o16", tag="o16",
                                    bufs=1)
                    # last chunk: run the rope combine on the Pool engine
                    # so DVE is free for phase 2's mask-adds immediately
                    ve = nc.vector
                    for j in range(8):
                        ve.tensor_mul(s1b[:, j, :], pcs[j], ss_sb)
                        ve.tensor_mul(pcs[j], pcs[j], cs_sb)
                    # in-place rotate-half swap of s1b (loose deadline, SWDGE)
                    for r in (0, 4):
                        lo = s1b[0:64, r:r + 4, :]
                        hi = s1b[64:128, r:r + 4, :]
                        nc.gpsimd.dma_start(out=tmp, in_=lo)
                        nc.gpsimd.dma_start(out=lo, in_=hi)
                        nc.gpsimd.dma_start(out=hi, in_=tmp)
                    for j in range(8):
                        ve.tensor_add(o16[:, j, :], pcs[j], s1b[:, j, :])
                    nc.gpsimd.dma_start(out=qdr.ap()[:, :, b, s0:s0 + 512],
                                        in_=o16[:, 0:4, :])
                    nc.gpsimd.dma_start(out=kdr.ap()[:, :, b, s0:s0 + 512],
                                        in_=o16[:, 4:8, :])
                    if ch == 3:
                        # b=0 q/k spilled; prefetch head-block 0 (vT
                        # follows at phase-2 entry from the qkv pool)
                        load_hb(0, tiles, hb0p, skip_v=True)

      # ------------- Phases 2+3 (w1 closed; weights SBUF freed) -------
      _p23(nc, tc, it, causal, mk_sb if causal else mk_d, wo_d, load_hb,
           tiles, hb0p, vdr[0].ap()[:, :, 0:128], out_d, F32, F32R, F16, EXP)


def _p23(nc, tc, it, causal, mk, wo_d, load_hb, tiles, hb0p, v0src, out_d,
         F32, F32R, F16, EXP):
    with (
        tc.tile_pool(name=f"p2c{it}", bufs=1) as p2c,
        tc.tile_pool(name=f"hoT{it}", bufs=1) as hop,
        tc.tile_pool(name=f"qkv{it}", bufs=2) as qkv,
    ):
        ones_sb = hb0p.tile([128, 1], F32R, tag="ones")
        nc.vector.memset(ones_sb.bitcast(F32), 1.0)
        wo_sb = p2c.tile([128, HLOC, D], F16, tag="wo")
        # hb0's vT (deferred from the phase-A prefetch)
        qT0, kT0, _ = tiles[0]
        vT0 = qkv.tile([128, KT, 128], F32R, name=f"vT{it}_0", tag="vT")
        nc.scalar.dma_start(out=vT0, in_=v0src)
        tiles[0] = (qT0, kT0, vT0)
        load_hb(1, tiles, qkv)

        hoTs = {}
        with (
            tc.tile_pool(name=f"sm{it}", bufs=2) as sm,
            tc.tile_pool(name=f"ps3{it}", bufs=2, space="PSUM") as ps3,
            tc.tile_pool(name=f"ps4{it}", bufs=4, space="PSUM") as ps4,
        ):
            for i in range(B * HLOC):
                b, h = divmod(i, HLOC)
                if h == 0:
                    hoTs[b] = hop.tile([128, HLOC, S], F16, name=f"hoT{it}_{b}",
                                       tag=f"hoT{b}")
                hoT = hoTs[b]
                if i + 2 < B * HLOC:
                    load_hb(i + 2, tiles, qkv)
                if 1 <= i <= HLOC:
                    # stream a quarter of wo in per head-block (deferred
                    # past the phase boundary's DMA burst)
                    nc.gpsimd.dma_start(out=wo_sb[:, i - 1, :],
                                        in_=wo_d.ap()[:, i - 1, :])
                qT, kT, vT = tiles.pop(i)
                for qc in (3, 0, 2, 1):
                    qs = qc * 512
                    nkt = (qc + 1) * 4 if causal else KT
                    sums = ps3.tile([1, 512], F32, name="sums", tag="sums")
                    hops = ps3.tile([128, 512], F32, name="hops", tag="hops")
                    # software pipeline: score matmul for k-tile kt issues
                    # before sums/PV for kt-1 so the PE never waits on exp
                    exs = [None] * nkt
                    for kt in range(nkt):
                        st = ps4.tile([128, 512], F32, name="st", tag="st")
                        nc.tensor.matmul(
                            st, kT[:, kt * 128:(kt + 1) * 128],
                            qT[:, qs:qs + 512],
                            start=True, stop=True,
                        )
                        if kt > 0:
                            pex = exs[kt - 1]
                            nc.tensor.matmul(sums, ones_sb, pex,
                                             start=(kt == 1), stop=False)
                            nc.tensor.matmul(hops, vT[:, kt - 1, :], pex,
                                             start=(kt == 1), stop=False)
                        if causal:
                            if kt >= nkt - 4:
                                nc.vector.tensor_add(
                                    st, st, mk[:, kt - (nkt - 4), :])
                        else:
                            mkt = sm.tile([128, 512], F32, name="mkt", tag="mkt")
                            nc.sync.dma_start(out=mkt,
                                              in_=mk.ap()[kt, :, qs:qs + 512])
                            nc.vector.tensor_add(st, st, mkt)
                        ex = hb0p.tile([128, 512], F32R, name="ex", tag="ex",
                                       bufs=3)
                        nc.scalar.activation(ex, st, EXP, scale=ISQRT)
                        exs[kt] = ex
                    pex = exs[nkt - 1]
                    nc.tensor.matmul(sums, ones_sb, pex,
                                     start=(nkt == 1), stop=True)
                    nc.tensor.matmul(hops, vT[:, nkt - 1, :], pex,
                                     start=(nkt == 1), stop=True)
                    recip = hb0p.tile([1, 512], F32, name="recip",
                                      tag="recip", bufs=1)
                    nc.vector.reciprocal(recip, sums)
                    bc = hb0p.tile([128, 512], F32, name="bc", tag="bc",
                                   bufs=1)
                    nc.gpsimd.partition_broadcast(bc, recip)
                    nc.vector.tensor_mul(hoT[:, h, qs:qs + 512], hops, bc)

        for b in range(B):
            _p3(nc, tc, it, b, hoTs[b], wo_sb, out_d, F32)


def _p3(nc, tc, it, b, hoT, wo_sb, out_d, F32):
    """Output projection for one batch: out[tok,:] = sum_h hoT_h' @ wo_h."""
    with (
        tc.tile_pool(name=f"oc{it}_{b}", bufs=3) as ocp,
        tc.tile_pool(name=f"ps5{it}_{b}", bufs=3, space="PSUM") as ps5,
    ):
        for t in range(S // 128):
            for oc2 in range(D // 1024):
                ot = ocp.tile([128, 1024], F32, name="ot", tag="ot")
                for half in range(2):
                    oc = 2 * oc2 + half
                    ops = ps5.tile([128, 512], F32, name="ops", tag=f"ops{half}")
                    for h in range(HLOC):
                        nc.tensor.matmul(
                            ops, hoT[:, h, t * 128:(t + 1) * 128],
                            wo_sb[:, h, oc * 512:(oc + 1) * 512],
                            start=(h == 0), stop=(h == HLOC - 1),
                        )
                    if half == 0:
                        nc.vector.tensor_copy(ot[:, 0:512], ops)
                    else:
                        nc.scalar.copy(ot[:, 512:1024], ops)
                nc.sync.dma_start(
                    out=out_d.ap()[
                        b * S + t * 128:b * S + (t + 1) * 128,
                        oc2 * 1024:(oc2 + 1) * 1024,
                    ],
                    in_=ot,
                )


def _get_nc(causal: bool):
    if causal not in _CACHE:
        _CACHE[causal] = _build(causal)
    return _CACHE[causal]


def _host_prep(x, wq, wk, wv, wo, freqs_cos, freqs_sin, mask):
    """Build per-core input maps."""
    x2 = np.ascontiguousarray(x.reshape(TOK, D).T)          # [D, TOK]
    xt = x2.reshape(DKT, 128, TOK)
    xt16 = xt.astype(np.float16)

    css = np.empty((128, 2, S), dtype=np.float16)
    css[:, 0, :] = np.concatenate([freqs_cos.T, freqs_cos.T], axis=0)
    css[:, 1, :] = np.concatenate([freqs_sin.T, -freqs_sin.T], axis=0)

    m2 = np.asarray(mask, dtype=np.float32).reshape(S, S)
    # causal iff: zero on/below diagonal, <= -1e8 strictly above
    tril = np.tril(np.ones((S, S), dtype=bool))
    causal = bool(np.all(m2[tril] == 0.0) and np.all(m2[~tril] <= -1e8))
    if causal:
        mk = np.ascontiguousarray(
            np.where(m2[:512, :512] < 0.0, np.float16(-30000), np.float16(0.0))
            .T.reshape(4, 128, 512).transpose(1, 0, 2)
        )
    else:
        mk = np.ascontiguousarray(m2.T.reshape(KT, 128, S))

    # per-head column permutation: evens then odds (RoPE rotate-half form)
    perm = np.concatenate([np.arange(0, HD, 2), np.arange(1, HD, 2)])

    in_maps = []
    for c in range(NC):
        cols = np.concatenate(
            [(4 * c + h) * HD + perm for h in range(HLOC)]
        )
        wq_c = np.ascontiguousarray(
            wq[:, cols].reshape(DKT, 128, 512).transpose(1, 0, 2)
        )
        wk_c = np.ascontiguousarray(
            wk[:, cols].reshape(DKT, 128, 512).transpose(1, 0, 2)
        )
        vcols = np.arange(4 * c * HD, 4 * (c + 1) * HD)
        wv_c = np.ascontiguousarray(
            wv[:, vcols].reshape(DKT, 128, 512).transpose(1, 0, 2)
        ).astype(np.float16)
        wo_c = np.ascontiguousarray(
            wo[vcols, :].reshape(HLOC, 128, D).transpose(1, 0, 2)
        ).astype(np.float16)
        m = {
            "xt": xt, "xt16": xt16, "wq": wq_c, "wk": wk_c, "wv16": wv_c,
            "wo16": wo_c, "css": css,
        }
        m["maskd" if causal else "maskf"] = mk
        in_maps.append(m)
    return in_maps, causal


def kernel(x, wq, wk, wv, wo, freqs_cos, freqs_sin, mask, **_unused):
    from concourse.bass_utils import run_bass_kernel_spmd

    x = np.asarray(x, dtype=np.float32)
    wq = np.asarray(wq, dtype=np.float32)
    wk = np.asarray(wk, dtype=np.float32)
    wv = np.asarray(wv, dtype=np.float32)
    wo = np.asarray(wo, dtype=np.float32)
    freqs_cos = np.asarray(freqs_cos, dtype=np.float32)
    freqs_sin = np.asarray(freqs_sin, dtype=np.float32)

    in_maps, causal = _host_prep(x, wq, wk, wv, wo, freqs_cos, freqs_sin, mask)
    nc = _get_nc(causal)
    res = run_bass_kernel_spmd(nc, in_maps, list(range(NC)))
    out = res.results[0]["out"]
    for c in range(1, NC):
        out = out + res.results[c]["out"]
    return out.reshape(B, S, D).astype(np.float32)
